# revision 1
# baseline (speedup 1.0000x reference)
"""MetaOptNet SVM-CS head on 8 Trainium2 NeuronCores.

Math: the reference runs a 15-iteration Mehrotra interior-point solve of the
Crammer-Singer dual QP per task. Empirically (f64 replication) the IPM is
fully converged by iteration 15, so the target equals the QP optimum. We
compute that optimum with a fixed-matrix ADMM:

    per task:  K = S S^T  (25x25 Gram)
               W~ = rho * (K + (1+rho) I)^{-1}   (Newton-Schulz, 3 iters:
                   2 in bf16 + 1 fp32 polish; |I - cH| <= ~0.1 since
                   9 <= eig(K+9I) <= ~17, and the final fp32 iteration
                   squares the bf16 error away)
               10x ADMM (rho=8), in (d1 = u-y, oy = y+oh/rho) state form:
                   t = center_ways(W~ @ d1) + oy
                   d1' = min(t, 2h - t);  oy' = max(t - (h - oh/rho), oh/rho)
                   where h = (C + 1/rho) oh
               logits = Q @ (S^T x) * scale    (x = center_ways(W~ @ d1))

The equality constraint A z = 0 (sum over ways per sample) reduces to
centering across ways because A A^T = n_way I; the KKT matrix is way-block-
diagonal with identical blocks K + (1+rho)I, which is what makes the single
25x25 inverse per task sufficient.

Sharding: pure data parallel, 16 tasks per core. Host-side work is layout
only (shard, transpose packing into 128-partition DMA tiles, one-hot
constants); all FLOPs run on-device.

Precision: the QP (Gram, inverse, ADMM) runs in fp32. S/S^T/Q^T are shipped
and contracted in bf16 — the error enters the output only linearly and
measures ~3e-3 relative on the logits (tolerance 2e-2); in exchange the
kernel is DMA-bound at roughly its memory floor (~35us cost-model time per
core vs ~33us of pure HBM traffic).

Tasks sit in 32-aligned 25-row partition blocks (PE tile_position constraint),
four tasks per 128-partition tile; the zero padding rides through every
matmul/elementwise op harmlessly.
"""

import sys

sys.path.insert(0, "/opt/trn_rl_repo")

from contextlib import ExitStack

import numpy as np

import concourse.bass as bass
import concourse.tile as tile
from concourse import mybir
from concourse.alu_op_type import AluOpType
from concourse.bass_utils import run_bass_kernel_spmd
from concourse.tile import TileContext

# ---------------------------------------------------------------------------
# Problem constants (hardcoded per the harness contract)
N_CORES = 8
B_TOT = 128
T = 16            # tasks per core
NS = 25           # support samples per task
NW = 5            # ways
NQ = 75           # queries per task
D = 2560          # feature dim
NCH = D // 128    # 20 d-chunks
G = 4             # task groups per core (4 tasks each -> 100-partition tiles)
GP = T // G       # tasks per group
RHO = 8.0
NS_C = 0.065      # Newton-Schulz init scale for H = K + 9I
NS_ITERS = 3
ADMM_ITERS = 10
C_REG = 0.1

F32 = mybir.dt.float32
BF16 = mybir.dt.bfloat16


# ---------------------------------------------------------------------------
# The walrus build here encodes at most ONE sync-wait command per instruction
# (TPB_CTRL / S3_LW setupSyncWait raises "Too many sync wait commands").
# Tile's scheduler freely attaches several waits to one instruction, so after
# scheduling we split the excess onto NoOps inserted immediately before the
# instruction on the same engine — identical semantics, encodable waits.
def _split_waits(nc, max_waits=1):
    cnt = 0
    for blk in nc.m.functions[0].blocks:
        insns = blk.instructions
        idx = 0
        while idx < len(insns):
            ins = insns[idx]
            si = ins.sync_info
            waits = list(si.on_wait) if si and si.on_wait else []
            if len(waits) > max_waits:
                si.on_wait = waits[:max_waits]
                for w in waits[max_waits:]:
                    nop = mybir.InstNoOp(name=f"waitnop_{cnt}", ins=[], outs=[])
                    cnt += 1
                    nop.engine = ins.engine
                    nop.sync_info = mybir.SyncInfo(on_wait=[w], on_update=[])
                    nc.register_instruction(nop, overwrite=True)
                    insns.insert(idx, nop)
                    idx += 1
            idx += 1
    return cnt


# ---------------------------------------------------------------------------
def _build_program():
    nc = bass.Bass("TRN2", target_bir_lowering=False)

    st_d = nc.dram_tensor("st", [NCH, 128, T * NS], BF16, kind="ExternalInput")
    sn_d = nc.dram_tensor("sn", [G, 128, D], BF16, kind="ExternalInput")
    qt_d = nc.dram_tensor("qt", [NCH, 128, T * NQ], BF16, kind="ExternalInput")
    ohc_d = nc.dram_tensor("ohc", [128, 20], F32, kind="ExternalInput")
    h2_d = nc.dram_tensor("h2", [128, 20], F32, kind="ExternalInput")
    hmo_d = nc.dram_tensor("hmo", [128, 20], F32, kind="ExternalInput")
    i2_d = nc.dram_tensor("i2", [128, 128], F32, kind="ExternalInput")
    cib_d = nc.dram_tensor("cib", [128, 128], BF16, kind="ExternalInput")
    nine_d = nc.dram_tensor("nine", [128, 128], F32, kind="ExternalInput")
    scale_d = nc.dram_tensor("scale", [1, 1], F32, kind="ExternalInput")
    out_d = nc.dram_tensor("out", [NQ, T * NW], F32, kind="ExternalOutput")

    with ExitStack() as ctx:
        tc = ctx.enter_context(TileContext(nc))
        st_pool = ctx.enter_context(tc.tile_pool(name="st", bufs=1))
        sn_pool = ctx.enter_context(tc.tile_pool(name="sn", bufs=G))
        qt_pool = ctx.enter_context(tc.tile_pool(name="qt", bufs=NCH))
        consts = ctx.enter_context(tc.tile_pool(name="consts", bufs=1))
        mats = ctx.enter_context(tc.tile_pool(name="mats", bufs=12))
        state = ctx.enter_context(tc.tile_pool(name="state", bufs=10))
        wout = ctx.enter_context(tc.tile_pool(name="wout", bufs=4))

        # ---- loads --------------------------------------------------------
        # NS-critical consts first on the Pool queue
        i2_sb = consts.tile([128, 128], F32, tag="i2")
        nc.gpsimd.dma_start(out=i2_sb, in_=i2_d[:, :])
        cib_sb = consts.tile([128, 128], BF16, tag="cib")
        nc.gpsimd.dma_start(out=cib_sb, in_=cib_d[:, :])
        nine_sb = consts.tile([128, 128], F32, tag="nine")
        nc.gpsimd.dma_start(out=nine_sb, in_=nine_d[:, :])
        st_tile = st_pool.tile([128, NCH * T * NS], BF16, tag="st")
        for j in range(4):
            nch4 = NCH // 4
            eng = nc.sync if j % 2 == 0 else nc.gpsimd
            eng.dma_start(
                out=st_tile[:, j * nch4 * T * NS : (j + 1) * nch4 * T * NS],
                in_=st_d[j * nch4 : (j + 1) * nch4, :, :],
            )
        st_sb = [
            st_tile[:, c * T * NS : (c + 1) * T * NS] for c in range(NCH)
        ]
        sn_sb = []
        for g in range(G):
            t_ = sn_pool.tile([128, D], BF16, tag="sn")
            nc.sync.dma_start(out=t_, in_=sn_d[g, :, :])
            sn_sb.append(t_)
        ohc_sb = consts.tile([128, 20], F32, tag="ohc")
        nc.gpsimd.dma_start(out=ohc_sb, in_=ohc_d[:, :])
        h2_sb = consts.tile([128, 20], F32, tag="h2")
        nc.gpsimd.dma_start(out=h2_sb, in_=h2_d[:, :])
        hmo_sb = consts.tile([128, 20], F32, tag="hmo")
        nc.gpsimd.dma_start(out=hmo_sb, in_=hmo_d[:, :])
        scale_sb = consts.tile([NQ, 1], F32, tag="scale")
        nc.gpsimd.dma_start(out=scale_sb, in_=scale_d[:, :].to_broadcast([NQ, 1]))

        # ADMM state: d1 = u - y (init ohc), oy = y + ohc (init ohc)
        d1_sb = state.tile([128, 20], F32, tag="d1")
        nc.gpsimd.dma_start(out=d1_sb, in_=ohc_d[:, :])
        oy_sb = state.tile([128, 20], F32, tag="oy")
        nc.gpsimd.dma_start(out=oy_sb, in_=ohc_d[:, :])

        qt_sb = []
        for c in range(NCH):
            t_ = qt_pool.tile([128, T * NQ], BF16, tag="qt")
            if c < NCH // 2:
                nc.gpsimd.dma_start(out=t_, in_=qt_d[c, :, :])
            qt_sb.append(t_)


        # ---- stage 1: K = S S^T, block-diagonal per 4-task group ----------
        h_all = []
        with tc.tile_pool(name="kpsum", bufs=4, space="PSUM") as kpsum:
            for g in range(G):
                kp = kpsum.tile([128, 128], F32, tag="kp")
                nc.vector.memset(kp, 0.0)
                for c in range(NCH):
                    for tp in range(GP):
                        t = g * GP + tp
                        sl = slice(tp * 32, tp * 32 + NS)
                        tsl = slice(t * NS, (t + 1) * NS)
                        nc.tensor.matmul(
                            kp[sl, sl],
                            lhsT=st_sb[c][:, tsl],
                            rhs=st_sb[c][:, tsl],
                            start=(c == 0),
                            stop=(c == NCH - 1),
                            tile_position=(0, tp * 32),
                        )
                h_sb = mats.tile([128, 128], F32, tag="h")
                nc.vector.tensor_tensor(h_sb, kp, nine_sb, op=AluOpType.add)
                h_all.append(h_sb)

        # ---- stage 2: Newton-Schulz inverse, 4 groups pipelined -----------
        # iters 0-1 in bf16 (NS self-corrects), final iter fp32 squares the
        # bf16 error away (~0.4%^2), so W~ is fp32-quality at 1/4 PE cost.
        wt_sb = []
        with tc.tile_pool(name="npsum", bufs=4, space="PSUM") as npsum:
            hb_all = []
            for g in range(G):
                hb = mats.tile([128, 128], BF16, tag="hb")
                nc.vector.tensor_copy(hb, h_all[g])
                hb_all.append(hb)
            x_cur = [cib_sb] * G
            for it in range(NS_ITERS):
                last = it == NS_ITERS - 1
                prev_last = it == NS_ITERS - 2
                for g in range(G):
                    t1p = npsum.tile([128, 128], F32, tag="t1p")
                    if last:
                        nc.tensor.matmul(
                            t1p, lhsT=h_all[g], rhs=x_cur[g], start=True, stop=True
                        )
                    else:
                        nc.tensor.matmul(
                            t1p, lhsT=hb_all[g], rhs=x_cur[g], start=True, stop=True
                        )
                    u_ns = mats.tile(
                        [128, 128], F32 if last else BF16,
                        tag="u_ns" if last else "u_nsb",
                    )
                    nc.vector.tensor_tensor(u_ns, i2_sb, t1p, op=AluOpType.subtract)
                    x2p = npsum.tile([128, 128], F32, tag="x2p")
                    nc.tensor.matmul(
                        x2p, lhsT=x_cur[g], rhs=u_ns, start=True, stop=True
                    )
                    if last:
                        wt = mats.tile([128, 128], F32, tag="wt")
                        nc.scalar.activation(
                            wt, x2p, mybir.ActivationFunctionType.Copy, scale=RHO
                        )
                        wt_sb.append(wt)
                    else:
                        x_next = mats.tile(
                            [128, 128], F32 if prev_last else BF16,
                            tag="x_ns" if prev_last else "x_nsb",
                        )
                        nc.scalar.activation(
                            x_next, x2p, mybir.ActivationFunctionType.Copy
                        )
                        x_cur[g] = x_next

        # ---- stage 3: ADMM (d1/oy state form) -----------------------------
        # t = center(Wt @ d1) + y + OHC;  d1' = min(t, 2h-t);  oy' = max(t-(h-OHC), OHC)
        xb_sb = None
        mpsum = ctx.enter_context(tc.tile_pool(name="mpsum", bufs=2, space="PSUM"))
        wpsum = ctx.enter_context(tc.tile_pool(name="wpsum", bufs=3, space="PSUM"))
        lpsum = ctx.enter_context(tc.tile_pool(name="lpsum", bufs=3, space="PSUM"))
        for it in range(ADMM_ITERS):
            xp = mpsum.tile([128, 20], F32, tag="mp")
            for g in range(G):
                nc.tensor.matmul(
                    xp[:, g * NW : (g + 1) * NW],
                    lhsT=wt_sb[g],
                    rhs=d1_sb[:, g * NW : (g + 1) * NW],
                    start=True,
                    stop=True,
                )
            msum = state.tile([128, 4], F32, tag="msum")
            nc.vector.reduce_sum(
                msum,
                xp[:, :].rearrange("p (g w) -> p g w", w=NW),
                axis=mybir.AxisListType.X,
            )
            msb = msum[:, :]
            msb_ap = bass.AP(
                tensor=msb.tensor, offset=msb.offset, ap=[msb.ap[0], msb.ap[1], [0, NW]]
            )
            p1 = state.tile([128, 20], F32, tag="p1")
            nc.vector.tensor_tensor(p1, xp, oy_sb, op=AluOpType.add)
            tt_sb = state.tile([128, 20], F32, tag="tt")
            nc.vector.scalar_tensor_tensor(
                out=tt_sb[:, :].rearrange("p (g w) -> p g w", w=NW),
                in0=msb_ap,
                scalar=-1.0 / NW,
                in1=p1[:, :].rearrange("p (g w) -> p g w", w=NW),
                op0=AluOpType.mult,
                op1=AluOpType.add,
            )
            if it == ADMM_ITERS - 1:
                xb_sb = state.tile([128, 20], BF16, tag="xb")
                nc.vector.scalar_tensor_tensor(
                    out=xb_sb[:, :].rearrange("p (g w) -> p g w", w=NW),
                    in0=msb_ap,
                    scalar=-1.0 / NW,
                    in1=xp[:, :].rearrange("p (g w) -> p g w", w=NW),
                    op0=AluOpType.mult,
                    op1=AluOpType.add,
                )
            n2h = state.tile([128, 20], F32, tag="n2h")
            nc.vector.scalar_tensor_tensor(
                out=n2h,
                in0=tt_sb,
                scalar=-1.0,
                in1=h2_sb,
                op0=AluOpType.mult,
                op1=AluOpType.add,
            )
            d1_sb = state.tile([128, 20], F32, tag="d1n")
            nc.vector.tensor_tensor(d1_sb, tt_sb, n2h, op=AluOpType.min)
            if it < ADMM_ITERS - 1:
                pa = state.tile([128, 20], F32, tag="pa")
                nc.vector.tensor_tensor(pa, tt_sb, hmo_sb, op=AluOpType.subtract)
                oy_sb = state.tile([128, 20], F32, tag="oy2")
                nc.vector.tensor_tensor(oy_sb, pa, ohc_sb, op=AluOpType.max)

        for c in range(NCH // 2, NCH):
            nc.gpsimd.dma_start(out=qt_sb[c], in_=qt_d[c, :, :])

        # ---- stages 4+5: w = S^T x ; logits = Q @ w (bf16 inputs) ----------
        out_sb = consts.tile([NQ, T * NW], F32, tag="outsb")
        for t in range(T):
            g, tp = t // GP, t % GP
            psl = slice(tp * 32, tp * 32 + NS)
            wp = wpsum.tile([128, NCH * NW], F32, tag="wp")
            for c in range(NCH):
                nc.tensor.matmul(
                    wp[:, c * NW : (c + 1) * NW],
                    lhsT=sn_sb[g][psl, c * 128 : (c + 1) * 128],
                    rhs=xb_sb[psl, g * NW : (g + 1) * NW],
                    start=True,
                    stop=True,
                    tile_position=(tp * 32, 0),
                )
            w_sb = wout.tile([128, NCH * NW], BF16, tag="w")
            nc.vector.tensor_copy(w_sb, wp)
            lp = lpsum.tile([NQ, NW], F32, tag="lp")
            for c in range(NCH):
                nc.tensor.matmul(
                    lp,
                    lhsT=qt_sb[c][:, t * NQ : (t + 1) * NQ],
                    rhs=w_sb[:, c * NW : (c + 1) * NW],
                    start=(c == 0),
                    stop=(c == NCH - 1),
                )
            nc.scalar.activation(
                out_sb[:, t * NW : (t + 1) * NW],
                lp,
                mybir.ActivationFunctionType.Copy,
                scale=scale_sb,
            )
        nc.sync.dma_start(out=out_d[:, :], in_=out_sb)

    _split_waits(nc)
    return nc


_NC_CACHE = None


def _get_nc():
    global _NC_CACHE
    if _NC_CACHE is None:
        _NC_CACHE = _build_program()
    return _NC_CACHE


# ---------------------------------------------------------------------------
def _host_prep(support, query, support_labels, scale):
    """Shard + pack into the DMA layouts. Layout only, no FLOPs."""
    f32 = np.float32
    eye = np.eye(NS, dtype=f32)
    blockdiag = np.zeros((128, 128), dtype=f32)
    for tp in range(GP):
        blockdiag[tp * 32 : tp * 32 + NS, tp * 32 : tp * 32 + NS] = eye
    i2 = np.ascontiguousarray(2.0 * blockdiag, dtype=f32)
    ci = np.ascontiguousarray(NS_C * blockdiag, dtype=f32)
    nine = np.ascontiguousarray((1.0 + RHO) * blockdiag, dtype=f32)
    sc = np.asarray(scale, dtype=f32).reshape(1, 1)

    in_maps = []
    for core in range(N_CORES):
        sl = slice(core * T, (core + 1) * T)
        S = np.asarray(support[sl], dtype=f32)        # [16,25,2560]
        Q = np.asarray(query[sl], dtype=f32)          # [16,75,2560]
        lab = np.asarray(support_labels[sl])          # [16,25] int
        st = np.ascontiguousarray(
            S.transpose(2, 0, 1).reshape(NCH, 128, T * NS).astype(mybir.dt.np(BF16))
        )
        bf = mybir.dt.np(BF16)
        sn = np.zeros((G, 128, D), dtype=bf)
        for tp in range(GP):
            sn[:, tp * 32 : tp * 32 + NS, :] = S.reshape(G, GP, NS, D)[:, tp].astype(bf)
        qt = np.ascontiguousarray(
            Q.transpose(2, 0, 1).reshape(NCH, 128, T * NQ).astype(mybir.dt.np(BF16))
        )
        oh = (lab[:, :, None] == np.arange(NW)[None, None, :]).astype(f32)
        # [16,25,5] -> [100,20]: row = tp*25+s, col = g*5+w
        ohm = np.zeros((128, 20), dtype=f32)
        ohr = oh.reshape(G, GP, NS, NW).transpose(1, 2, 0, 3).reshape(GP, NS, 20)
        for tp in range(GP):
            ohm[tp * 32 : tp * 32 + NS, :] = ohr[tp]
        in_maps.append(
            {
                "st": st,
                "sn": sn,
                "qt": qt,
                "ohc": np.ascontiguousarray(ohm / RHO),
                "h2": np.ascontiguousarray(2.0 * (C_REG + 1.0 / RHO) * ohm),
                "hmo": np.ascontiguousarray(C_REG * ohm),
                "i2": i2,
                "cib": np.ascontiguousarray(ci.astype(mybir.dt.np(BF16))),
                "nine": nine,
                "scale": sc,
            }
        )
    return in_maps


def kernel(query, support, scale, support_labels, n_way, n_shot):
    assert int(n_way) == NW and int(n_shot) * int(n_way) == NS
    assert query.shape == (B_TOT, NQ, D) and support.shape == (B_TOT, NS, D)
    nc = _get_nc()
    in_maps = _host_prep(support, query, support_labels, scale)
    res = run_bass_kernel_spmd(nc, in_maps, core_ids=list(range(N_CORES)))
    outs = []
    for core in range(N_CORES):
        o = np.asarray(res.results[core]["out"])      # [75, 80]
        outs.append(o.reshape(NQ, T, NW).transpose(1, 0, 2))
    return np.ascontiguousarray(np.concatenate(outs, axis=0), dtype=np.float32)



# revision 6
# speedup vs baseline: 1.9570x; 1.9570x over previous
"""MetaOptNet SVM-CS head on 8 Trainium2 NeuronCores.

Math: the reference runs a 15-iteration Mehrotra interior-point solve of the
Crammer-Singer dual QP per task. Empirically (f64 replication) the IPM is
fully converged by iteration 15, so the target equals the QP optimum. We
compute that optimum with a fixed-matrix ADMM:

    per task:  K = S S^T  (25x25 Gram)
               W~ = rho * (K + (1+rho) I)^{-1}   (Newton-Schulz, 3 iters:
                   2 in bf16 + 1 fp32 polish; |I - cH| <= ~0.1 since
                   9 <= eig(K+9I) <= ~17, and the final fp32 iteration
                   squares the bf16 error away)
               10x ADMM (rho=8), in (d1 = u-y, oy = y+oh/rho) state form:
                   t = center_ways(W~ @ d1) + oy
                   d1' = min(t, 2h - t);  oy' = max(t - (h - oh/rho), oh/rho)
                   where h = (C + 1/rho) oh
               logits = Q @ (S^T x) * scale    (x = center_ways(W~ @ d1))

The equality constraint A z = 0 (sum over ways per sample) reduces to
centering across ways because A A^T = n_way I; the KKT matrix is way-block-
diagonal with identical blocks K + (1+rho)I, which is what makes the single
25x25 inverse per task sufficient.

Sharding: pure data parallel, 16 tasks per core. Host-side work is layout
only (shard, transpose packing into 128-partition DMA tiles, one-hot
constants); all FLOPs run on-device.

Precision: the QP (Gram, inverse, ADMM) runs in fp32. S/S^T/Q^T are shipped
and contracted in bf16 — the error enters the output only linearly and
measures ~3e-3 relative on the logits (tolerance 2e-2); in exchange the
kernel is DMA-bound at roughly its memory floor (~35us cost-model time per
core vs ~33us of pure HBM traffic).

Tasks sit in 32-aligned 25-row partition blocks (PE tile_position constraint),
four tasks per 128-partition tile; the zero padding rides through every
matmul/elementwise op harmlessly.
"""

import sys

sys.path.insert(0, "/opt/trn_rl_repo")

from contextlib import ExitStack

import numpy as np

import concourse.bass as bass
import concourse.tile as tile
from concourse import mybir
from concourse.alu_op_type import AluOpType
from concourse.bass_utils import run_bass_kernel_spmd
from concourse.tile import TileContext

# ---------------------------------------------------------------------------
# Problem constants (hardcoded per the harness contract)
N_CORES = 8
B_TOT = 128
T = 16            # tasks per core
NS = 25           # support samples per task
NW = 5            # ways
NQ = 75           # queries per task
D = 2560          # feature dim
NCH = D // 128    # 20 d-chunks
G = 4             # task groups per core (4 tasks each -> 100-partition tiles)
GP = T // G       # tasks per group
RHO = 8.0
NS_C = 0.065      # Newton-Schulz init scale for H = K + 9I
NS_ITERS = 3
ADMM_ITERS = 10
C_REG = 0.1

F32 = mybir.dt.float32
BF16 = mybir.dt.bfloat16

# Packed-input column offsets. All bf16 payloads (S^T chunks, S row-blocks,
# Q^T chunks) ride in one [128, DATA_COLS] tensor; all f32 constants in one
# [128, CPACK_COLS] tensor. One DRAM tensor per dtype keeps the per-dispatch
# buffer-binding count (and the host->device staging surface) minimal.
ST0 = 0                       # S^T: NCH blocks of T*NS cols
SN0 = ST0 + NCH * T * NS      # S row-blocks: G blocks of D cols
QT0 = SN0 + G * D             # Q^T: NCH blocks of T*NQ cols
DATA_COLS = QT0 + NCH * T * NQ
OHC0, H20, HMO0, BD0, SCL0 = 0, 20, 40, 60, 188
CPACK_COLS = 189


# ---------------------------------------------------------------------------
# The walrus build here encodes at most ONE sync-wait command per instruction
# (TPB_CTRL / S3_LW setupSyncWait raises "Too many sync wait commands").
# Tile's scheduler freely attaches several waits to one instruction, so after
# scheduling we split the excess onto NoOps inserted immediately before the
# instruction on the same engine — identical semantics, encodable waits.
def _split_waits(nc, max_waits=1):
    cnt = 0
    for blk in nc.m.functions[0].blocks:
        insns = blk.instructions
        idx = 0
        while idx < len(insns):
            ins = insns[idx]
            si = ins.sync_info
            waits = list(si.on_wait) if si and si.on_wait else []
            if len(waits) > max_waits:
                si.on_wait = waits[:max_waits]
                for w in waits[max_waits:]:
                    nop = mybir.InstNoOp(name=f"waitnop_{cnt}", ins=[], outs=[])
                    cnt += 1
                    nop.engine = ins.engine
                    nop.sync_info = mybir.SyncInfo(on_wait=[w], on_update=[])
                    nc.register_instruction(nop, overwrite=True)
                    insns.insert(idx, nop)
                    idx += 1
            idx += 1
    return cnt


# ---------------------------------------------------------------------------
def _build_program():
    nc = bass.Bass("TRN2", target_bir_lowering=False)

    data_d = nc.dram_tensor("data", [128, DATA_COLS], BF16, kind="ExternalInput")
    cpack_d = nc.dram_tensor("cpack", [128, CPACK_COLS], F32, kind="ExternalInput")
    out_d = nc.dram_tensor("out", [NQ, T * NW], F32, kind="ExternalOutput")

    with ExitStack() as ctx:
        tc = ctx.enter_context(TileContext(nc))
        st_pool = ctx.enter_context(tc.tile_pool(name="st", bufs=1))
        sn_pool = ctx.enter_context(tc.tile_pool(name="sn", bufs=G))
        qt_pool = ctx.enter_context(tc.tile_pool(name="qt", bufs=NCH))
        consts = ctx.enter_context(tc.tile_pool(name="consts", bufs=1))
        mats = ctx.enter_context(tc.tile_pool(name="mats", bufs=12))
        state = ctx.enter_context(tc.tile_pool(name="state", bufs=10))
        wout = ctx.enter_context(tc.tile_pool(name="wout", bufs=4))

        # ---- loads --------------------------------------------------------
        # NS-critical consts first on the Pool queue; i2/nine/cib are scalar
        # multiples of the block-diagonal mask, derived on the idle Act engine
        bd_sb = consts.tile([128, 128], F32, tag="bd")
        nc.gpsimd.dma_start(out=bd_sb, in_=cpack_d[:, BD0 : BD0 + 128])
        i2_sb = consts.tile([128, 128], F32, tag="i2")
        nc.scalar.activation(i2_sb, bd_sb, mybir.ActivationFunctionType.Copy, scale=2.0)
        cib_sb = consts.tile([128, 128], BF16, tag="cib")
        nc.scalar.activation(
            cib_sb, bd_sb, mybir.ActivationFunctionType.Copy, scale=NS_C
        )
        nine_sb = consts.tile([128, 128], F32, tag="nine")
        nc.scalar.activation(
            nine_sb, bd_sb, mybir.ActivationFunctionType.Copy, scale=1.0 + RHO
        )
        st_tile = st_pool.tile([128, NCH * T * NS], BF16, tag="st")
        for j in range(4):
            q = NCH * T * NS // 4
            eng = nc.sync if j % 2 == 0 else nc.gpsimd
            eng.dma_start(
                out=st_tile[:, j * q : (j + 1) * q],
                in_=data_d[:, ST0 + j * q : ST0 + (j + 1) * q],
            )
        st_sb = [
            st_tile[:, c * T * NS : (c + 1) * T * NS] for c in range(NCH)
        ]
        sn_sb = []
        for g in range(G):
            t_ = sn_pool.tile([128, D], BF16, tag="sn")
            nc.sync.dma_start(out=t_, in_=data_d[:, SN0 + g * D : SN0 + (g + 1) * D])
            sn_sb.append(t_)
        ohc_sb = consts.tile([128, 20], F32, tag="ohc")
        nc.gpsimd.dma_start(out=ohc_sb, in_=cpack_d[:, OHC0 : OHC0 + 20])
        h2_sb = consts.tile([128, 20], F32, tag="h2")
        nc.gpsimd.dma_start(out=h2_sb, in_=cpack_d[:, H20 : H20 + 20])
        hmo_sb = consts.tile([128, 20], F32, tag="hmo")
        nc.gpsimd.dma_start(out=hmo_sb, in_=cpack_d[:, HMO0 : HMO0 + 20])
        scale_sb = consts.tile([NQ, 1], F32, tag="scale")
        nc.gpsimd.dma_start(
            out=scale_sb,
            in_=cpack_d[0:1, SCL0 : SCL0 + 1].to_broadcast([NQ, 1]),
        )

        # ADMM state: d1 = u - y (init ohc), oy = y + ohc (init ohc)
        d1_sb = state.tile([128, 20], F32, tag="d1")
        nc.gpsimd.dma_start(out=d1_sb, in_=cpack_d[:, OHC0 : OHC0 + 20])
        oy_sb = state.tile([128, 20], F32, tag="oy")
        nc.gpsimd.dma_start(out=oy_sb, in_=cpack_d[:, OHC0 : OHC0 + 20])

        qt_sb = []
        for c in range(NCH):
            t_ = qt_pool.tile([128, T * NQ], BF16, tag="qt")
            if c < NCH // 2:
                nc.gpsimd.dma_start(
                    out=t_, in_=data_d[:, QT0 + c * T * NQ : QT0 + (c + 1) * T * NQ]
                )
            qt_sb.append(t_)


        # ---- stage 1: K = S S^T, block-diagonal per 4-task group ----------
        h_all = []
        with tc.tile_pool(name="kpsum", bufs=4, space="PSUM") as kpsum:
            for g in range(G):
                kp = kpsum.tile([128, 128], F32, tag="kp")
                nc.vector.memset(kp, 0.0)
                for c in range(NCH):
                    for tp in range(GP):
                        t = g * GP + tp
                        sl = slice(tp * 32, tp * 32 + NS)
                        tsl = slice(t * NS, (t + 1) * NS)
                        nc.tensor.matmul(
                            kp[sl, sl],
                            lhsT=st_sb[c][:, tsl],
                            rhs=st_sb[c][:, tsl],
                            start=(c == 0),
                            stop=(c == NCH - 1),
                            tile_position=(0, tp * 32),
                        )
                h_sb = mats.tile([128, 128], F32, tag="h")
                nc.vector.tensor_tensor(h_sb, kp, nine_sb, op=AluOpType.add)
                h_all.append(h_sb)

        # ---- stage 2: Newton-Schulz inverse, 4 groups pipelined -----------
        # iters 0-1 in bf16 (NS self-corrects), final iter fp32 squares the
        # bf16 error away (~0.4%^2), so W~ is fp32-quality at 1/4 PE cost.
        wt_sb = []
        with tc.tile_pool(name="npsum", bufs=4, space="PSUM") as npsum:
            hb_all = []
            for g in range(G):
                hb = mats.tile([128, 128], BF16, tag="hb")
                nc.vector.tensor_copy(hb, h_all[g])
                hb_all.append(hb)
            x_cur = [cib_sb] * G
            for it in range(NS_ITERS):
                last = it == NS_ITERS - 1
                prev_last = it == NS_ITERS - 2
                for g in range(G):
                    t1p = npsum.tile([128, 128], F32, tag="t1p")
                    if last:
                        nc.tensor.matmul(
                            t1p, lhsT=h_all[g], rhs=x_cur[g], start=True, stop=True
                        )
                    else:
                        nc.tensor.matmul(
                            t1p, lhsT=hb_all[g], rhs=x_cur[g], start=True, stop=True
                        )
                    u_ns = mats.tile(
                        [128, 128], F32 if last else BF16,
                        tag="u_ns" if last else "u_nsb",
                    )
                    nc.vector.tensor_tensor(u_ns, i2_sb, t1p, op=AluOpType.subtract)
                    x2p = npsum.tile([128, 128], F32, tag="x2p")
                    nc.tensor.matmul(
                        x2p, lhsT=x_cur[g], rhs=u_ns, start=True, stop=True
                    )
                    if last:
                        wt = mats.tile([128, 128], F32, tag="wt")
                        nc.scalar.activation(
                            wt, x2p, mybir.ActivationFunctionType.Copy, scale=RHO
                        )
                        wt_sb.append(wt)
                    else:
                        x_next = mats.tile(
                            [128, 128], F32 if prev_last else BF16,
                            tag="x_ns" if prev_last else "x_nsb",
                        )
                        nc.scalar.activation(
                            x_next, x2p, mybir.ActivationFunctionType.Copy
                        )
                        x_cur[g] = x_next

        # ---- stage 3: ADMM (d1/oy state form) -----------------------------
        # t = center(Wt @ d1) + y + OHC;  d1' = min(t, 2h-t);  oy' = max(t-(h-OHC), OHC)
        xb_sb = None
        mpsum = ctx.enter_context(tc.tile_pool(name="mpsum", bufs=2, space="PSUM"))
        wpsum = ctx.enter_context(tc.tile_pool(name="wpsum", bufs=3, space="PSUM"))
        lpsum = ctx.enter_context(tc.tile_pool(name="lpsum", bufs=3, space="PSUM"))
        for it in range(ADMM_ITERS):
            xp = mpsum.tile([128, 20], F32, tag="mp")
            for g in range(G):
                nc.tensor.matmul(
                    xp[:, g * NW : (g + 1) * NW],
                    lhsT=wt_sb[g],
                    rhs=d1_sb[:, g * NW : (g + 1) * NW],
                    start=True,
                    stop=True,
                )
            msum = state.tile([128, 4], F32, tag="msum")
            nc.vector.reduce_sum(
                msum,
                xp[:, :].rearrange("p (g w) -> p g w", w=NW),
                axis=mybir.AxisListType.X,
            )
            msb = msum[:, :]
            msb_ap = bass.AP(
                tensor=msb.tensor, offset=msb.offset, ap=[msb.ap[0], msb.ap[1], [0, NW]]
            )
            p1 = state.tile([128, 20], F32, tag="p1")
            nc.vector.tensor_tensor(p1, xp, oy_sb, op=AluOpType.add)
            tt_sb = state.tile([128, 20], F32, tag="tt")
            nc.vector.scalar_tensor_tensor(
                out=tt_sb[:, :].rearrange("p (g w) -> p g w", w=NW),
                in0=msb_ap,
                scalar=-1.0 / NW,
                in1=p1[:, :].rearrange("p (g w) -> p g w", w=NW),
                op0=AluOpType.mult,
                op1=AluOpType.add,
            )
            if it == ADMM_ITERS - 1:
                xb_sb = state.tile([128, 20], BF16, tag="xb")
                nc.vector.scalar_tensor_tensor(
                    out=xb_sb[:, :].rearrange("p (g w) -> p g w", w=NW),
                    in0=msb_ap,
                    scalar=-1.0 / NW,
                    in1=xp[:, :].rearrange("p (g w) -> p g w", w=NW),
                    op0=AluOpType.mult,
                    op1=AluOpType.add,
                )
            n2h = state.tile([128, 20], F32, tag="n2h")
            nc.vector.scalar_tensor_tensor(
                out=n2h,
                in0=tt_sb,
                scalar=-1.0,
                in1=h2_sb,
                op0=AluOpType.mult,
                op1=AluOpType.add,
            )
            d1_sb = state.tile([128, 20], F32, tag="d1n")
            nc.vector.tensor_tensor(d1_sb, tt_sb, n2h, op=AluOpType.min)
            if it < ADMM_ITERS - 1:
                pa = state.tile([128, 20], F32, tag="pa")
                nc.vector.tensor_tensor(pa, tt_sb, hmo_sb, op=AluOpType.subtract)
                oy_sb = state.tile([128, 20], F32, tag="oy2")
                nc.vector.tensor_tensor(oy_sb, pa, ohc_sb, op=AluOpType.max)

        for c in range(NCH // 2, NCH):
            nc.gpsimd.dma_start(
                out=qt_sb[c], in_=data_d[:, QT0 + c * T * NQ : QT0 + (c + 1) * T * NQ]
            )

        # ---- stages 4+5: w = S^T x ; logits = Q @ w (bf16 inputs) ----------
        out_sb = consts.tile([NQ, T * NW], F32, tag="outsb")
        for t in range(T):
            g, tp = t // GP, t % GP
            psl = slice(tp * 32, tp * 32 + NS)
            wp = wpsum.tile([128, NCH * NW], F32, tag="wp")
            for c in range(NCH):
                nc.tensor.matmul(
                    wp[:, c * NW : (c + 1) * NW],
                    lhsT=sn_sb[g][psl, c * 128 : (c + 1) * 128],
                    rhs=xb_sb[psl, g * NW : (g + 1) * NW],
                    start=True,
                    stop=True,
                    tile_position=(tp * 32, 0),
                )
            w_sb = wout.tile([128, NCH * NW], BF16, tag="w")
            nc.vector.tensor_copy(w_sb, wp)
            lp = lpsum.tile([NQ, NW], F32, tag="lp")
            for c in range(NCH):
                nc.tensor.matmul(
                    lp,
                    lhsT=qt_sb[c][:, t * NQ : (t + 1) * NQ],
                    rhs=w_sb[:, c * NW : (c + 1) * NW],
                    start=(c == 0),
                    stop=(c == NCH - 1),
                )
            nc.scalar.activation(
                out_sb[:, t * NW : (t + 1) * NW],
                lp,
                mybir.ActivationFunctionType.Copy,
                scale=scale_sb,
            )
        nc.sync.dma_start(out=out_d[:, :], in_=out_sb)

    _split_waits(nc)
    return nc


_NC_CACHE = None


def _get_nc():
    global _NC_CACHE
    if _NC_CACHE is None:
        _NC_CACHE = _build_program()
    return _NC_CACHE


# ---------------------------------------------------------------------------
def _host_prep(support, query, support_labels, scale):
    """Shard + pack into the two DMA tensors. Layout only, no FLOPs."""
    f32 = np.float32
    bf = mybir.dt.np(BF16)
    eye = np.eye(NS, dtype=f32)
    blockdiag = np.zeros((128, 128), dtype=f32)
    for tp in range(GP):
        blockdiag[tp * 32 : tp * 32 + NS, tp * 32 : tp * 32 + NS] = eye

    in_maps = []
    for core in range(N_CORES):
        sl = slice(core * T, (core + 1) * T)
        S = np.asarray(support[sl], dtype=f32)        # [16,25,2560]
        Q = np.asarray(query[sl], dtype=f32)          # [16,75,2560]
        lab = np.asarray(support_labels[sl])          # [16,25] int
        data = np.empty((128, DATA_COLS), dtype=bf)
        # S^T chunks: col block c holds S[:, :, c*128:(c+1)*128]^T as [128, T*NS]
        data[:, ST0:SN0] = (
            S.transpose(2, 0, 1).reshape(NCH, 128, T * NS)
            .transpose(1, 0, 2).reshape(128, NCH * T * NS).astype(bf)
        )
        # S row-blocks: 4 groups, tasks at 32-aligned partition offsets
        sn = np.zeros((G, 128, D), dtype=bf)
        for tp in range(GP):
            sn[:, tp * 32 : tp * 32 + NS, :] = S.reshape(G, GP, NS, D)[:, tp].astype(bf)
        data[:, SN0:QT0] = sn.transpose(1, 0, 2).reshape(128, G * D)
        # Q^T chunks
        data[:, QT0:] = (
            Q.transpose(2, 0, 1).reshape(NCH, 128, T * NQ)
            .transpose(1, 0, 2).reshape(128, NCH * T * NQ).astype(bf)
        )
        oh = (lab[:, :, None] == np.arange(NW)[None, None, :]).astype(f32)
        # [16,25,5] -> [100,20]: row = tp*25+s, col = g*5+w
        ohm = np.zeros((128, 20), dtype=f32)
        ohr = oh.reshape(G, GP, NS, NW).transpose(1, 2, 0, 3).reshape(GP, NS, 20)
        for tp in range(GP):
            ohm[tp * 32 : tp * 32 + NS, :] = ohr[tp]
        cpack = np.zeros((128, CPACK_COLS), dtype=f32)
        cpack[:, OHC0 : OHC0 + 20] = ohm / RHO
        cpack[:, H20 : H20 + 20] = 2.0 * (C_REG + 1.0 / RHO) * ohm
        cpack[:, HMO0 : HMO0 + 20] = C_REG * ohm
        cpack[:, BD0 : BD0 + 128] = blockdiag
        cpack[0, SCL0] = np.asarray(scale, dtype=f32).reshape(-1)[0]
        in_maps.append({"data": data, "cpack": cpack})
    return in_maps


def kernel(query, support, scale, support_labels, n_way, n_shot):
    assert int(n_way) == NW and int(n_shot) * int(n_way) == NS
    assert query.shape == (B_TOT, NQ, D) and support.shape == (B_TOT, NS, D)
    nc = _get_nc()
    in_maps = _host_prep(support, query, support_labels, scale)
    res = run_bass_kernel_spmd(nc, in_maps, core_ids=list(range(N_CORES)))
    outs = []
    for core in range(N_CORES):
        o = np.asarray(res.results[core]["out"])      # [75, 80]
        outs.append(o.reshape(NQ, T, NW).transpose(1, 0, 2))
    return np.ascontiguousarray(np.concatenate(outs, axis=0), dtype=np.float32)



# revision 20
# speedup vs baseline: 2.1794x; 1.1136x over previous
"""MetaOptNet SVM-CS head on 8 Trainium2 NeuronCores.

Math: the reference runs a 15-iteration Mehrotra interior-point solve of the
Crammer-Singer dual QP per task. Empirically (f64 replication) the IPM is
fully converged by iteration 15, so the target equals the QP optimum. We
compute that optimum with a fixed-matrix ADMM:

    per task:  K = S S^T  (25x25 Gram)
               W~ = rho * (K + (1+rho) I)^{-1}   (Newton-Schulz, 3 iters:
                   2 in bf16 + 1 fp32 polish; |I - cH| <= ~0.1 since
                   9 <= eig(K+9I) <= ~17, and the final fp32 iteration
                   squares the bf16 error away)
               10x ADMM (rho=8), in (d1 = u-y, oy = y+oh/rho) state form:
                   t = center_ways(W~ @ d1) + oy
                   d1' = min(t, 2h - t);  oy' = max(t - (h - oh/rho), oh/rho)
                   where h = (C + 1/rho) oh
               logits = (S Q^T)^T x * scale    (x = center_ways(W~ @ d1))

The equality constraint A z = 0 (sum over ways per sample) reduces to
centering across ways because A A^T = n_way I; the KKT matrix is way-block-
diagonal with identical blocks K + (1+rho)I, which is what makes the single
25x25 inverse per task sufficient.

Sharding: pure data parallel, 16 tasks per core. Host-side work is layout
only (shard, transpose packing into 128-partition DMA tiles, one-hot
constants); all FLOPs run on-device. All inputs ride in two DRAM tensors
(one bf16 payload, one f32 const pack) to minimize per-dispatch buffer
bindings.

Precision: the QP (Gram, inverse, ADMM) runs in fp32. S^T/Q^T are shipped
and contracted in bf16 — the error enters the output only linearly and
measures ~4e-3 relative on the logits (tolerance 2e-2). The epilogue
contracts C = S Q^T over d (both operands d-major), so S is shipped once;
HBM traffic is the irreducible S + Q + consts (~8.2 MB/core), and the
kernel is DMA-bound at that floor (~26us cost-model time per core).

Tasks sit in 32-aligned 25-row partition blocks (PE tile_position constraint),
four tasks per 128-partition tile; the zero padding rides through every
matmul/elementwise op harmlessly.
"""

import sys

sys.path.insert(0, "/opt/trn_rl_repo")

from contextlib import ExitStack

import numpy as np

import concourse.bass as bass
import concourse.tile as tile
from concourse import mybir
from concourse.alu_op_type import AluOpType
from concourse.bass_utils import run_bass_kernel_spmd
from concourse.tile import TileContext

# ---------------------------------------------------------------------------
# Problem constants (hardcoded per the harness contract)
N_CORES = 8
B_TOT = 128
T = 16            # tasks per core
NS = 25           # support samples per task
NW = 5            # ways
NQ = 75           # queries per task
D = 2560          # feature dim
NCH = D // 128    # 20 d-chunks
G = 4             # task groups per core (4 tasks each -> 100-partition tiles)
GP = T // G       # tasks per group
RHO = 8.0
NS_C = 0.065      # Newton-Schulz init scale for H = K + 9I
NS_ITERS = 3
ADMM_ITERS = 10
C_REG = 0.1

F32 = mybir.dt.float32
BF16 = mybir.dt.bfloat16

# Packed-input column offsets. All bf16 payloads (S^T chunks, S row-blocks,
# Q^T chunks) ride in one [128, DATA_COLS] tensor; all f32 constants in one
# [128, CPACK_COLS] tensor. One DRAM tensor per dtype keeps the per-dispatch
# buffer-binding count (and the host->device staging surface) minimal.
ST0 = 0                       # S^T: NCH blocks of T*NS cols
QT0 = ST0 + NCH * T * NS      # Q^T: NCH blocks of T*NQ cols
DATA_COLS = QT0 + NCH * T * NQ
OHC0, H20, HMO0, BD0, SCL0 = 0, 20, 40, 60, 188
CPACK_COLS = 189


# ---------------------------------------------------------------------------
# The walrus build here encodes at most ONE sync-wait command per instruction
# (TPB_CTRL / S3_LW setupSyncWait raises "Too many sync wait commands").
# Tile's scheduler freely attaches several waits to one instruction, so after
# scheduling we split the excess onto NoOps inserted immediately before the
# instruction on the same engine — identical semantics, encodable waits.
def _split_waits(nc, max_waits=1):
    cnt = 0
    for blk in nc.m.functions[0].blocks:
        insns = blk.instructions
        idx = 0
        while idx < len(insns):
            ins = insns[idx]
            si = ins.sync_info
            waits = list(si.on_wait) if si and si.on_wait else []
            if len(waits) > max_waits:
                si.on_wait = waits[:max_waits]
                for w in waits[max_waits:]:
                    nop = mybir.InstNoOp(name=f"waitnop_{cnt}", ins=[], outs=[])
                    cnt += 1
                    nop.engine = ins.engine
                    nop.sync_info = mybir.SyncInfo(on_wait=[w], on_update=[])
                    nc.register_instruction(nop, overwrite=True)
                    insns.insert(idx, nop)
                    idx += 1
            idx += 1
    return cnt


# ---------------------------------------------------------------------------
def _build_program():
    nc = bass.Bass("TRN2", target_bir_lowering=False)

    data_d = nc.dram_tensor("data", [128, DATA_COLS], BF16, kind="ExternalInput")
    cpack_d = nc.dram_tensor("cpack", [128, CPACK_COLS], F32, kind="ExternalInput")
    out_d = nc.dram_tensor("out", [NQ, T * NW], F32, kind="ExternalOutput")

    with ExitStack() as ctx:
        tc = ctx.enter_context(TileContext(nc))
        st_pool = ctx.enter_context(tc.tile_pool(name="st", bufs=1))
        qt_pool = ctx.enter_context(tc.tile_pool(name="qt", bufs=NCH))
        consts = ctx.enter_context(tc.tile_pool(name="consts", bufs=1))
        mats = ctx.enter_context(tc.tile_pool(name="mats", bufs=12))
        state = ctx.enter_context(tc.tile_pool(name="state", bufs=10))
        wout = ctx.enter_context(tc.tile_pool(name="wout", bufs=4))

        # ---- loads --------------------------------------------------------
        # NS-critical consts first on the Pool queue; i2/nine/cib are scalar
        # multiples of the block-diagonal mask, derived on the idle Act engine
        bd_sb = consts.tile([128, 128], F32, tag="bd")
        nc.gpsimd.dma_start(out=bd_sb, in_=cpack_d[:, BD0 : BD0 + 128])
        i2_sb = consts.tile([128, 128], F32, tag="i2")
        nc.scalar.activation(i2_sb, bd_sb, mybir.ActivationFunctionType.Copy, scale=2.0)
        cib_sb = consts.tile([128, 128], BF16, tag="cib")
        nc.scalar.activation(
            cib_sb, bd_sb, mybir.ActivationFunctionType.Copy, scale=NS_C
        )
        nine_sb = consts.tile([128, 128], F32, tag="nine")
        nc.scalar.activation(
            nine_sb, bd_sb, mybir.ActivationFunctionType.Copy, scale=1.0 + RHO
        )
        st_tile = st_pool.tile([128, NCH * T * NS], BF16, tag="st")
        for j in range(4):
            q = NCH * T * NS // 4
            eng = nc.sync if j % 2 == 0 else nc.gpsimd
            eng.dma_start(
                out=st_tile[:, j * q : (j + 1) * q],
                in_=data_d[:, ST0 + j * q : ST0 + (j + 1) * q],
            )
        st_sb = [
            st_tile[:, c * T * NS : (c + 1) * T * NS] for c in range(NCH)
        ]
        ohc_sb = consts.tile([128, 20], F32, tag="ohc")
        nc.gpsimd.dma_start(out=ohc_sb, in_=cpack_d[:, OHC0 : OHC0 + 20])
        h2_sb = consts.tile([128, 20], F32, tag="h2")
        nc.gpsimd.dma_start(out=h2_sb, in_=cpack_d[:, H20 : H20 + 20])
        hmo_sb = consts.tile([128, 20], F32, tag="hmo")
        nc.gpsimd.dma_start(out=hmo_sb, in_=cpack_d[:, HMO0 : HMO0 + 20])
        scale_sb = consts.tile([NQ, 1], F32, tag="scale")
        nc.gpsimd.dma_start(
            out=scale_sb,
            in_=cpack_d[0:1, SCL0 : SCL0 + 1].to_broadcast([NQ, 1]),
        )

        # ADMM state: d1 = u - y (init ohc), oy = y + ohc (init ohc)
        d1_sb = state.tile([128, 20], F32, tag="d1")
        nc.gpsimd.dma_start(out=d1_sb, in_=cpack_d[:, OHC0 : OHC0 + 20])
        oy_sb = state.tile([128, 20], F32, tag="oy")
        nc.gpsimd.dma_start(out=oy_sb, in_=cpack_d[:, OHC0 : OHC0 + 20])

        # all Q^T chunks loaded up front, interleaved across both DMA queues
        qt_sb = []
        for c in range(NCH):
            t_ = qt_pool.tile([128, T * NQ], BF16, tag="qt")
            eng = nc.sync if c % 2 == 0 else nc.gpsimd
            eng.dma_start(
                out=t_, in_=data_d[:, QT0 + c * T * NQ : QT0 + (c + 1) * T * NQ]
            )
            qt_sb.append(t_)


        # ---- stage 1: K = S S^T, block-diagonal per 4-task group ----------
        h_all = []
        with tc.tile_pool(name="kpsum", bufs=4, space="PSUM") as kpsum:
            for g in range(G):
                kp = kpsum.tile([128, 128], F32, tag="kp")
                nc.vector.memset(kp, 0.0)
                for c in range(NCH):
                    for tp in range(GP):
                        t = g * GP + tp
                        sl = slice(tp * 32, tp * 32 + NS)
                        tsl = slice(t * NS, (t + 1) * NS)
                        nc.tensor.matmul(
                            kp[sl, sl],
                            lhsT=st_sb[c][:, tsl],
                            rhs=st_sb[c][:, tsl],
                            start=(c == 0),
                            stop=(c == NCH - 1),
                            tile_position=(0, tp * 32),
                        )
                h_sb = mats.tile([128, 128], F32, tag="h")
                nc.vector.tensor_tensor(h_sb, kp, nine_sb, op=AluOpType.add)
                h_all.append(h_sb)

        # ---- stage 2: Newton-Schulz inverse, 4 groups pipelined -----------
        # iters 0-1 in bf16 (NS self-corrects), final iter fp32 squares the
        # bf16 error away (~0.4%^2), so W~ is fp32-quality at 1/4 PE cost.
        wt_sb = []
        with tc.tile_pool(name="npsum", bufs=4, space="PSUM") as npsum:
            hb_all = []
            for g in range(G):
                hb = mats.tile([128, 128], BF16, tag="hb")
                nc.vector.tensor_copy(hb, h_all[g])
                hb_all.append(hb)
            x_cur = [cib_sb] * G
            for it in range(NS_ITERS):
                last = it == NS_ITERS - 1
                prev_last = it == NS_ITERS - 2
                for g in range(G):
                    t1p = npsum.tile([128, 128], F32, tag="t1p")
                    if last:
                        nc.tensor.matmul(
                            t1p, lhsT=h_all[g], rhs=x_cur[g], start=True, stop=True
                        )
                    else:
                        nc.tensor.matmul(
                            t1p, lhsT=hb_all[g], rhs=x_cur[g], start=True, stop=True
                        )
                    u_ns = mats.tile(
                        [128, 128], F32 if last else BF16,
                        tag="u_ns" if last else "u_nsb",
                    )
                    nc.vector.tensor_tensor(u_ns, i2_sb, t1p, op=AluOpType.subtract)
                    x2p = npsum.tile([128, 128], F32, tag="x2p")
                    nc.tensor.matmul(
                        x2p, lhsT=x_cur[g], rhs=u_ns, start=True, stop=True
                    )
                    if last:
                        wt = mats.tile([128, 128], F32, tag="wt")
                        nc.scalar.activation(
                            wt, x2p, mybir.ActivationFunctionType.Copy, scale=RHO
                        )
                        wt_sb.append(wt)
                    else:
                        x_next = mats.tile(
                            [128, 128], F32 if prev_last else BF16,
                            tag="x_ns" if prev_last else "x_nsb",
                        )
                        nc.scalar.activation(
                            x_next, x2p, mybir.ActivationFunctionType.Copy
                        )
                        x_cur[g] = x_next

        # ---- stage 3: ADMM (d1/oy state form) -----------------------------
        # t = center(Wt @ d1) + y + OHC;  d1' = min(t, 2h-t);  oy' = max(t-(h-OHC), OHC)
        xb_sb = None
        mpsum = ctx.enter_context(tc.tile_pool(name="mpsum", bufs=2, space="PSUM"))
        lpsum = ctx.enter_context(tc.tile_pool(name="lpsum", bufs=2, space="PSUM"))
        for it in range(ADMM_ITERS):
            xp = mpsum.tile([128, 20], F32, tag="mp")
            for g in range(G):
                nc.tensor.matmul(
                    xp[:, g * NW : (g + 1) * NW],
                    lhsT=wt_sb[g],
                    rhs=d1_sb[:, g * NW : (g + 1) * NW],
                    start=True,
                    stop=True,
                )
            msum = state.tile([128, 4], F32, tag="msum")
            nc.vector.reduce_sum(
                msum,
                xp[:, :].rearrange("p (g w) -> p g w", w=NW),
                axis=mybir.AxisListType.X,
            )
            msb = msum[:, :]
            msb_ap = bass.AP(
                tensor=msb.tensor, offset=msb.offset, ap=[msb.ap[0], msb.ap[1], [0, NW]]
            )
            p1 = state.tile([128, 20], F32, tag="p1")
            nc.vector.tensor_tensor(p1, xp, oy_sb, op=AluOpType.add)
            tt_sb = state.tile([128, 20], F32, tag="tt")
            nc.vector.scalar_tensor_tensor(
                out=tt_sb[:, :].rearrange("p (g w) -> p g w", w=NW),
                in0=msb_ap,
                scalar=-1.0 / NW,
                in1=p1[:, :].rearrange("p (g w) -> p g w", w=NW),
                op0=AluOpType.mult,
                op1=AluOpType.add,
            )
            if it == ADMM_ITERS - 1:
                xb_sb = state.tile([128, 20], BF16, tag="xb")
                nc.vector.scalar_tensor_tensor(
                    out=xb_sb[:, :].rearrange("p (g w) -> p g w", w=NW),
                    in0=msb_ap,
                    scalar=-1.0 / NW,
                    in1=xp[:, :].rearrange("p (g w) -> p g w", w=NW),
                    op0=AluOpType.mult,
                    op1=AluOpType.add,
                )
            n2h = state.tile([128, 20], F32, tag="n2h")
            nc.vector.scalar_tensor_tensor(
                out=n2h,
                in0=tt_sb,
                scalar=-1.0,
                in1=h2_sb,
                op0=AluOpType.mult,
                op1=AluOpType.add,
            )
            d1_sb = state.tile([128, 20], F32, tag="d1n")
            nc.vector.tensor_tensor(d1_sb, tt_sb, n2h, op=AluOpType.min)
            if it < ADMM_ITERS - 1:
                pa = state.tile([128, 20], F32, tag="pa")
                nc.vector.tensor_tensor(pa, tt_sb, hmo_sb, op=AluOpType.subtract)
                oy_sb = state.tile([128, 20], F32, tag="oy2")
                nc.vector.tensor_tensor(oy_sb, pa, ohc_sb, op=AluOpType.max)

        # ---- stage 4: C = S Q^T per task (contract over d; st/qt are both
        # d-major, so no second copy of S is ever shipped) -------------------
        # One PSUM bank per task GROUP: matmul start=True clears has_written
        # for the written partitions across the whole bank, so accumulation
        # chains that share a bank must sit on disjoint partitions. Group g's
        # bank holds its four tasks as 32-aligned partition bands (like the
        # Gram stage); interleaving chains across banks is then safe and the
        # c-outer order lets C consume Q^T chunks as the DMAs land.
        cpsum = ctx.enter_context(tc.tile_pool(name="cpsum", bufs=G, space="PSUM"))
        cp_all = []
        for g in range(G):
            cp = cpsum.tile([128, NQ], F32, tag="cp")
            cp_all.append(cp)
        for c in range(NCH):
            for g in range(G):
                for tp in range(GP):
                    t = g * GP + tp
                    sl = slice(tp * 32, tp * 32 + NS)
                    nc.tensor.matmul(
                        cp_all[g][sl, :],
                        lhsT=st_sb[c][:, t * NS : (t + 1) * NS],
                        rhs=qt_sb[c][:, t * NQ : (t + 1) * NQ],
                        start=(c == 0),
                        stop=(c == NCH - 1),
                        tile_position=(0, tp * 32),
                    )

        # ---- stage 5: logits = C^T x, scaled ------------------------------
        out_sb = consts.tile([NQ, T * NW], F32, tag="outsb")
        for g in range(G):
            cb = wout.tile([128, NQ], BF16, tag="cb")
            nc.vector.tensor_copy(cb, cp_all[g])
            for tp in range(GP):
                t = g * GP + tp
                sl = slice(tp * 32, tp * 32 + NS)
                lp = lpsum.tile([NQ, NW], F32, tag="lp")
                nc.tensor.matmul(
                    lp,
                    lhsT=cb[sl, :],
                    rhs=xb_sb[sl, g * NW : (g + 1) * NW],
                    start=True,
                    stop=True,
                    tile_position=(tp * 32, 0),
                )
                nc.scalar.activation(
                    out_sb[:, t * NW : (t + 1) * NW],
                    lp,
                    mybir.ActivationFunctionType.Copy,
                    scale=scale_sb,
                )
        nc.sync.dma_start(out=out_d[:, :], in_=out_sb)

    _split_waits(nc)
    return nc


_NC_CACHE = None


def _get_nc():
    global _NC_CACHE
    if _NC_CACHE is None:
        _NC_CACHE = _build_program()
    return _NC_CACHE


# ---------------------------------------------------------------------------
def _host_prep(support, query, support_labels, scale):
    """Shard + pack into the two DMA tensors. Layout only, no FLOPs."""
    f32 = np.float32
    bf = mybir.dt.np(BF16)
    eye = np.eye(NS, dtype=f32)
    blockdiag = np.zeros((128, 128), dtype=f32)
    for tp in range(GP):
        blockdiag[tp * 32 : tp * 32 + NS, tp * 32 : tp * 32 + NS] = eye

    in_maps = []
    for core in range(N_CORES):
        sl = slice(core * T, (core + 1) * T)
        S = np.asarray(support[sl], dtype=f32)        # [16,25,2560]
        Q = np.asarray(query[sl], dtype=f32)          # [16,75,2560]
        lab = np.asarray(support_labels[sl])          # [16,25] int
        data = np.empty((128, DATA_COLS), dtype=bf)
        # S^T chunks: col block c holds S[:, :, c*128:(c+1)*128]^T as [128, T*NS]
        data[:, ST0:QT0] = (
            S.transpose(2, 0, 1).reshape(NCH, 128, T * NS)
            .transpose(1, 0, 2).reshape(128, NCH * T * NS).astype(bf)
        )
        # Q^T chunks
        data[:, QT0:] = (
            Q.transpose(2, 0, 1).reshape(NCH, 128, T * NQ)
            .transpose(1, 0, 2).reshape(128, NCH * T * NQ).astype(bf)
        )
        oh = (lab[:, :, None] == np.arange(NW)[None, None, :]).astype(f32)
        # [16,25,5] -> [100,20]: row = tp*25+s, col = g*5+w
        ohm = np.zeros((128, 20), dtype=f32)
        ohr = oh.reshape(G, GP, NS, NW).transpose(1, 2, 0, 3).reshape(GP, NS, 20)
        for tp in range(GP):
            ohm[tp * 32 : tp * 32 + NS, :] = ohr[tp]
        cpack = np.zeros((128, CPACK_COLS), dtype=f32)
        cpack[:, OHC0 : OHC0 + 20] = ohm / RHO
        cpack[:, H20 : H20 + 20] = 2.0 * (C_REG + 1.0 / RHO) * ohm
        cpack[:, HMO0 : HMO0 + 20] = C_REG * ohm
        cpack[:, BD0 : BD0 + 128] = blockdiag
        cpack[0, SCL0] = np.asarray(scale, dtype=f32).reshape(-1)[0]
        in_maps.append({"data": data, "cpack": cpack})
    return in_maps


def kernel(query, support, scale, support_labels, n_way, n_shot):
    assert int(n_way) == NW and int(n_shot) * int(n_way) == NS
    assert query.shape == (B_TOT, NQ, D) and support.shape == (B_TOT, NS, D)
    nc = _get_nc()
    in_maps = _host_prep(support, query, support_labels, scale)
    res = run_bass_kernel_spmd(nc, in_maps, core_ids=list(range(N_CORES)))
    outs = []
    for core in range(N_CORES):
        o = np.asarray(res.results[core]["out"])      # [75, 80]
        outs.append(o.reshape(NQ, T, NW).transpose(1, 0, 2))
    return np.ascontiguousarray(np.concatenate(outs, axis=0), dtype=np.float32)

